# revision 12
# baseline (speedup 1.0000x reference)
"""AttentionPool Trainium2 Bass kernel (v2: pair-reduced pooling).

Computes, for h:[N,512] f32, sorted batch_vec:[N] int, gate-MLP weights
W1/b1/W2/b2:
    gate  = gelu(h @ W1 + b1) @ W2 + b2            (erf gelu)
    alpha = segment_softmax(gate, batch_vec)       (1024 segments)
    out   = segment_sum(alpha[:,None] * h)         -> [1024, 512]

Sharding: data-parallel over graphs. Core c owns graphs [128c, 128c+128)
and its contiguous node range (batch_vec sorted => segments never
straddle cores).

Per-core design (all engines overlapped, DMA+PE near roofline):
- Gate phase (A): z^T = W1^T @ h^T per 512-node supertile on the tensor
  engine; merged gelu over d-chunk pairs on ACT; the small gate matmuls
  (a1 @ W2) for supertile s are DEFERRED to supertile s+1 so the PE
  never stalls waiting on ACT.
- Host reorders nodes so that same-graph node PAIRS sit in two aligned
  SBUF tiles; DVE computes wh = e0*h0 + e1*h1 (idle engine), which
  HALVES the pooling matmul stream on the PE. Odd leftover nodes
  (<=1 per graph) go through the classic e-weighted one-hot path in
  LT tile(s). exp runs in chunks every ES supertiles so pooling is
  interleaved into the gate phase; softmax max-subtraction is skipped
  (gates are O(1), mathematically identical).
- pooled = msp^T @ wh accumulated in PSUM; denom = msp^T @ [e0 e1];
  out = pooled / denom.
"""

import os
from contextlib import ExitStack, nullcontext

import numpy as np

import concourse.bass as bass
import concourse.mybir as mybir
from concourse import bacc
import concourse.tile as tile
from concourse.bass_utils import run_bass_kernel_spmd

F32 = mybir.dt.float32
F16 = mybir.dt.float16

N_NODES = 100000
H = 512
NUM_GRAPHS = 1024
N_CORES = 8
G = NUM_GRAPHS // N_CORES   # graphs per core = 128
NP_DEFAULT = 12800          # gate-path padded nodes (25 supertiles of 512)
KC = H // 128               # contraction chunks = 4

ES = int(os.environ.get("AP_ES", "4"))          # exp chunk every ES supertiles
HT_BUFS = int(os.environ.get("AP_HT_BUFS", "4"))
WH_BUFS = int(os.environ.get("AP_WH_BUFS", "8"))
A1_BUFS = int(os.environ.get("AP_A1_BUFS", "6"))


def _build(np_ht: int, PT: int, LT: int, reps: int = 1, ablate: str = "",
           merged: bool = True):
    """Build the per-core Bass program (SPMD: same program, per-core data)."""
    T = np_ht // 128
    S = np_ht // 512

    nc = bacc.Bacc("TRN2", target_bir_lowering=False, debug=False)

    ht_d = nc.dram_tensor("hT", [H, np_ht], F16, kind="ExternalInput")
    hpp_d = nc.dram_tensor("hpp", [PT * 256, H], F16, kind="ExternalInput")
    hps_d = nc.dram_tensor("hps", [LT * 128, H], F16, kind="ExternalInput")
    w1_d = nc.dram_tensor("W1", [H, H], F16, kind="ExternalInput")
    b1v_d = nc.dram_tensor("b1v", [128, KC], F32, kind="ExternalInput")
    w2_d = nc.dram_tensor("W2v", [128, 2 * KC], F16, kind="ExternalInput")
    b2t_d = nc.dram_tensor("b2t", [128, 1], F32, kind="ExternalInput")
    bv_d = nc.dram_tensor("bvrel", [128, T], F32, kind="ExternalInput")
    io_d = nc.dram_tensor("iota", [128, 128], F16, kind="ExternalInput")
    out_d = nc.dram_tensor("out", [G, H], F32, kind="ExternalOutput")

    gelu = mybir.ActivationFunctionType.Gelu
    expf = mybir.ActivationFunctionType.Exp
    eq = mybir.AluOpType.is_equal
    mul = mybir.AluOpType.mult
    addop = mybir.AluOpType.add

    with tile.TileContext(nc) as tc, ExitStack() as ctx:
        consts = ctx.enter_context(tc.tile_pool(name="consts", bufs=1))
        ht_pool = ctx.enter_context(tc.tile_pool(name="ht", bufs=HT_BUFS))
        a1_pool = ctx.enter_context(tc.tile_pool(name="a1", bufs=A1_BUFS))
        hpp_pool = ctx.enter_context(tc.tile_pool(name="hpp", bufs=PT))
        hps_pool = ctx.enter_context(tc.tile_pool(name="hps", bufs=LT))
        wh_pool = ctx.enter_context(tc.tile_pool(name="wh", bufs=WH_BUFS))
        msp_pool = ctx.enter_context(tc.tile_pool(name="msp", bufs=6))
        small = ctx.enter_context(tc.tile_pool(name="small", bufs=2))
        psz = ctx.enter_context(tc.tile_pool(name="psz", bufs=2, space="PSUM"))
        psg = ctx.enter_context(tc.tile_pool(name="psg", bufs=2, space="PSUM"))
        psp = ctx.enter_context(tc.tile_pool(name="psp", bufs=1, space="PSUM"))
        psd = ctx.enter_context(tc.tile_pool(name="psd", bufs=1, space="PSUM"))

        w1_sb = []
        for k in range(KC):
            t = consts.tile([128, H], F16, tag=f"w1_{k}")
            nc.sync.dma_start(out=t, in_=w1_d.ap()[k * 128:(k + 1) * 128, :])
            w1_sb.append(t)
        b1_sb = consts.tile([128, KC], F32, tag="b1")
        nc.sync.dma_start(out=b1_sb, in_=b1v_d.ap())
        w2_sb = consts.tile([128, 2 * KC], F16, tag="w2")
        nc.sync.dma_start(out=w2_sb, in_=w2_d.ap())
        b2_sb = consts.tile([128, 1], F32, tag="b2")
        nc.sync.dma_start(out=b2_sb, in_=b2t_d.ap())
        io_sb = consts.tile([128, 128], F16, tag="iota")
        nc.sync.dma_start(out=io_sb, in_=io_d.ap())
        bv_sb = consts.tile([128, T], F32, tag="bv")
        nc.sync.dma_start(out=bv_sb, in_=bv_d.ap())
        ones_sb = consts.tile([128, 2], F16, tag="ones")
        nc.vector.memset(ones_sb, 1.0)
        gate_sb = consts.tile([128, T], F32, tag="gate")
        e32 = consts.tile([128, T], F32, tag="e32")
        e16 = consts.tile([128, T], F16, tag="e16")

        ht4 = ht_d.ap().rearrange("(k p) (s n) -> s p k n", p=128, n=512)
        hpp4 = hpp_d.ap().rearrange("(pt i r) d -> pt r i d", i=2, r=128)
        hps3 = hps_d.ap().rearrange("(l r) d -> l r d", r=128)

        do_A = ablate not in ("noA", "dmaonly")
        do_C = ablate not in ("noC", "dmaonly")
        do_gate = ablate not in ("nogate", "noA", "dmaonly")

        loop_cm = tc.For_i(0, reps, 1) if reps > 1 else nullcontext()
        with loop_cm:
            if do_C:
                psp_t = psp.tile([128, H], F32, tag="pp")
                psd_t = psd.tile([128, 2], F32, tag="pd")
            if not do_gate:
                nc.vector.memset(e32, 0.125)
                nc.vector.memset(e16, 0.125)

            state = {"pp_first": True, "pt_dma": 0, "wh_done": 0,
                     "mm_done": 0, "e_cols": 0}
            hpp_tiles = {}
            hps_tiles = {}
            a1_tiles = {}
            wh_tiles = {}
            ms_tiles = {}

            def emit_pair_dmas(n):
                for _ in range(n):
                    if state["pt_dma"] >= PT:
                        return
                    tl = hpp_pool.tile([128, 2, H], F16, tag="hpp")
                    nc.sync.dma_start(out=tl, in_=hpp4[state["pt_dma"]])
                    hpp_tiles[state["pt_dma"]] = tl
                    state["pt_dma"] += 1

            def emit_pg(s):
                pg = psg.tile([128, 2 * KC], F32, tag="pg")
                for nch in range(4):
                    for d in range(KC):
                        g2, jj = divmod(d, 2)
                        nc.tensor.matmul(
                            out=pg[:, 2 * nch:2 * nch + 2],
                            lhsT=a1_tiles[s][g2][:, jj, nch * 128:(nch + 1) * 128],
                            rhs=w2_sb[:, 2 * d:2 * d + 2],
                            start=(d == 0), stop=(d == KC - 1))
                nc.vector.tensor_copy(out=gate_sb[:, 4 * s:4 * s + 4],
                                      in_=pg[:, 0:2 * KC:2])
                del a1_tiles[s]

            def emit_exp(upto_col):
                if upto_col <= state["e_cols"]:
                    return
                a, b = state["e_cols"], upto_col
                nc.scalar.activation(
                    out=e32[:, a:b], in_=gate_sb[:, a:b],
                    func=expf, bias=b2_sb[:, 0:1], scale=1.0)
                nc.vector.tensor_copy(out=e16[:, a:b], in_=e32[:, a:b])
                state["e_cols"] = upto_col

            def emit_C_wh(upto_pt):
                upto_pt = min(upto_pt, PT, state["e_cols"] // 2)
                for pt in range(state["wh_done"], upto_pt):
                    hb = hpp_tiles.pop(pt)
                    whE = wh_pool.tile([128, H], F16, tag="whE")
                    nc.vector.tensor_scalar(
                        out=whE, in0=hb[:, 0, :],
                        scalar1=e32[:, 2 * pt:2 * pt + 1], scalar2=None,
                        op0=mul)
                    whS = wh_pool.tile([128, H], F16, tag="whS")
                    nc.vector.scalar_tensor_tensor(
                        out=whS, in0=hb[:, 1, :],
                        scalar=e32[:, 2 * pt + 1:2 * pt + 2], in1=whE,
                        op0=mul, op1=addop)
                    mspt = msp_pool.tile([128, 128], F16, tag="msp")
                    nc.vector.tensor_scalar(
                        out=mspt, in0=io_sb,
                        scalar1=bv_sb[:, 2 * pt:2 * pt + 1], scalar2=None,
                        op0=eq)
                    wh_tiles[pt] = whS
                    ms_tiles[pt] = mspt
                state["wh_done"] = max(state["wh_done"], upto_pt)

            def emit_C_mm(upto_pt):
                upto_pt = min(upto_pt, state["wh_done"])
                for pt in range(state["mm_done"], upto_pt):
                    whS = wh_tiles.pop(pt)
                    mspt = ms_tiles.pop(pt)
                    first = state["pp_first"]
                    state["pp_first"] = False
                    nc.tensor.matmul(out=psp_t, lhsT=mspt, rhs=whS,
                                     start=first, stop=False)
                    nc.tensor.matmul(out=psd_t, lhsT=mspt,
                                     rhs=e16[:, 2 * pt:2 * pt + 2],
                                     start=first, stop=False)
                state["mm_done"] = upto_pt

            # ---------------- main supertile loop ----------------
            for s in range(S):
                htb = ht_pool.tile([128, KC, H], F16, tag="ht")
                nc.sync.dma_start(out=htb, in_=ht4[s])
                if s == 0:
                    for l in range(LT):
                        tl = hps_pool.tile([128, H], F16, tag="hps")
                        nc.sync.dma_start(out=tl, in_=hps3[l])
                        hps_tiles[l] = tl
                emit_pair_dmas(4 if s == 0 else 2)

                if do_A:
                    for g2 in range(2):
                        pz = psz.tile([128, 2, H], F32, tag="pz")
                        for jj in range(2):
                            d = 2 * g2 + jj
                            for k in range(KC):
                                nc.tensor.matmul(
                                    out=pz[:, jj, :],
                                    lhsT=w1_sb[k][:, d * 128:(d + 1) * 128],
                                    rhs=htb[:, k, :],
                                    start=(k == 0), stop=(k == KC - 1))
                        a1t = a1_pool.tile([128, 2, H], F16, tag="a1")
                        if merged:
                            nc.scalar.activation(out=a1t, in_=pz, func=gelu,
                                                 bias=b1_sb[:, 0:1], scale=1.0)
                        else:
                            for jj in range(2):
                                d = 2 * g2 + jj
                                nc.scalar.activation(
                                    out=a1t[:, jj, :], in_=pz[:, jj, :],
                                    func=gelu, bias=b1_sb[:, d:d + 1],
                                    scale=1.0)
                        a1_tiles.setdefault(s, {})[g2] = a1t
                    if do_gate and s >= 1:
                        emit_pg(s - 1)
                if do_gate and s > 0 and s % ES == 0:
                    emit_exp(4 * s)
                if do_C:
                    emit_C_mm(state["wh_done"])
                    emit_C_wh(min(state["wh_done"] + 3, 2 * s))

            # ---------------- tail ----------------
            if do_A and do_gate:
                emit_pg(S - 1)
                emit_exp(T)
            if do_C:
                emit_pair_dmas(PT)
                emit_C_wh(PT)
                emit_C_mm(PT)
                for l in range(LT):
                    msl = msp_pool.tile([128, 128], F16, tag="msl")
                    col = 2 * PT + l
                    nc.vector.tensor_scalar(
                        out=msl, in0=io_sb, scalar1=bv_sb[:, col:col + 1],
                        scalar2=e32[:, col:col + 1], op0=eq, op1=mul)
                    last = (l == LT - 1)
                    nc.tensor.matmul(out=psp_t, lhsT=msl, rhs=hps_tiles[l],
                                     start=False, stop=last)
                    nc.tensor.matmul(out=psd_t[:, 0:1], lhsT=msl,
                                     rhs=ones_sb[:, 0:1], start=False,
                                     stop=last)
                pdsb = small.tile([128, 2], F32, tag="pdsb")
                nc.vector.tensor_copy(out=pdsb, in_=psd_t)
                dsum = small.tile([128, 1], F32, tag="dsum")
                nc.vector.tensor_add(out=dsum, in0=pdsb[:, 0:1],
                                     in1=pdsb[:, 1:2])
                dcl = small.tile([128, 1], F32, tag="dcl")
                nc.vector.tensor_scalar(out=dcl, in0=dsum, scalar1=1e-35,
                                        scalar2=None,
                                        op0=mybir.AluOpType.max)
                rec = small.tile([128, 1], F32, tag="rec")
                nc.vector.reciprocal(out=rec, in_=dcl)
                osb = small.tile([128, H], F32, tag="osb")
                nc.vector.tensor_scalar(out=osb, in0=psp_t,
                                        scalar1=rec[:, 0:1], scalar2=None,
                                        op0=mul)
            else:
                osb = small.tile([128, H], F32, tag="osb")
                nc.vector.memset(osb, 0.0)
            nc.sync.dma_start(out=out_d.ap(), in_=osb)

    nc.compile()
    return nc


def _plan(bv: np.ndarray) -> dict:
    """Node reordering plan: same-graph pairs + leftover singles, per core."""
    bv = np.asarray(bv).astype(np.int64)
    bounds = np.searchsorted(bv, np.arange(0, NUM_GRAPHS + 1, G))
    cores = []
    for c in range(N_CORES):
        n0, n1 = int(bounds[c]), int(bounds[c + 1])
        rel = bv[n0:n1] - c * G
        lens = np.bincount(rel, minlength=G)
        ends = np.cumsum(lens)
        starts = ends - lens
        p0_list, singles = [], []
        for g in range(G):
            ln = int(lens[g])
            s0 = int(starts[g])
            if ln >= 2:
                p0_list.append(s0 + 2 * np.arange(ln // 2))
            if ln % 2:
                singles.append(s0 + ln - 1)
        p0 = (np.concatenate(p0_list) if p0_list
              else np.empty(0, np.int64))
        sg = np.asarray(singles, np.int64)
        cores.append({"n0": n0, "n1": n1, "rel": rel, "p0": p0, "sg": sg})
    PT = max(1, -(-max(len(cc["p0"]) for cc in cores) // 128))
    LT = max(1, -(-max(len(cc["sg"]) for cc in cores) // 128))
    np_pool = PT * 256 + LT * 128
    np_ht = max(NP_DEFAULT, -(-np_pool // 512) * 512)
    return {"PT": PT, "LT": LT, "np_ht": np_ht, "cores": cores}


def _merged(b1: np.ndarray) -> bool:
    return bool(np.all(b1 == b1.reshape(-1)[0]))


def _prep_in_maps(h, bv, W1, b1, W2, b2, plan):
    """Shard + reorder + pad inputs per core; list of per-core input dicts."""
    PT, LT, np_ht = plan["PT"], plan["LT"], plan["np_ht"]
    T = np_ht // 128

    w1f = np.ascontiguousarray(W1.astype(np.float16))
    b1v = np.ascontiguousarray(
        b1.astype(np.float32).reshape(KC, 128).T)
    w2v = np.zeros((128, 2 * KC), np.float16)
    w2v[:, 0::2] = W2[:, 0].astype(np.float16).reshape(KC, 128).T
    b2t = np.full((128, 1), np.float32(b2.reshape(-1)[0]), np.float32)
    iota = np.ascontiguousarray(
        np.tile(np.arange(128, dtype=np.float16), (128, 1)))

    in_maps = []
    for cc in plan["cores"]:
        hc = h[cc["n0"]:cc["n1"]]
        rel = cc["rel"]
        p0, sg = cc["p0"], cc["sg"]
        m0 = np.full(PT * 128, -1, np.int64)
        m0[:len(p0)] = p0
        m1 = np.full(PT * 128, -1, np.int64)
        m1[:len(p0)] = p0 + 1
        pair_src = np.stack(
            [m0.reshape(PT, 128), m1.reshape(PT, 128)], axis=1).reshape(-1)
        sg_src = np.full(LT * 128, -1, np.int64)
        sg_src[:len(sg)] = sg
        src = np.concatenate([pair_src, sg_src])
        npool = len(src)

        hpool = np.zeros((np_ht, H), np.float32)
        valid = src >= 0
        hpool[:npool][valid] = hc[src[valid]]
        bvs = np.full(np_ht, -1.0, np.float32)
        bvs[:npool][valid] = rel[src[valid]].astype(np.float32)

        in_maps.append({
            "hT": np.ascontiguousarray(hpool.T.astype(np.float16)),
            "hpp": np.ascontiguousarray(hpool[:PT * 256].astype(np.float16)),
            "hps": np.ascontiguousarray(
                hpool[PT * 256:PT * 256 + LT * 128].astype(np.float16)),
            "W1": w1f,
            "b1v": b1v,
            "W2v": w2v,
            "b2t": b2t,
            "bvrel": np.ascontiguousarray(bvs.reshape(T, 128).T),
            "iota": iota,
        })
    return in_maps


_prog_cache: dict = {}


def _get_prog(np_ht, PT, LT, merged, reps=1, ablate=""):
    key = (np_ht, PT, LT, merged, reps, ablate, ES)
    if key not in _prog_cache:
        _prog_cache[key] = _build(np_ht, PT, LT, reps=reps, ablate=ablate,
                                  merged=merged)
    return _prog_cache[key]


def kernel(**inputs) -> np.ndarray:
    h = np.ascontiguousarray(np.asarray(inputs["h"], dtype=np.float32))
    bv = np.asarray(inputs["batch_vec"]).astype(np.int64)
    W1 = np.asarray(inputs["W1"], dtype=np.float32)
    b1 = np.asarray(inputs["b1"], dtype=np.float32)
    W2 = np.asarray(inputs["W2"], dtype=np.float32)
    b2 = np.asarray(inputs["b2"], dtype=np.float32)

    plan = _plan(bv)
    nc = _get_prog(plan["np_ht"], plan["PT"], plan["LT"], _merged(b1))
    in_maps = _prep_in_maps(h, bv, W1, b1, W2, b2, plan)
    trace = bool(int(os.environ.get("AP_TRACE", "0")))
    res = run_bass_kernel_spmd(nc, in_maps, list(range(N_CORES)), trace=trace)
    global last_results
    last_results = res
    out = np.concatenate([res.results[c]["out"] for c in range(N_CORES)],
                         axis=0).astype(np.float32)
    return out


last_results = None


# revision 19
# speedup vs baseline: 1.0869x; 1.0869x over previous
"""AttentionPool Trainium2 Bass kernel (v2: pair-reduced pooling).

Computes, for h:[N,512] f32, sorted batch_vec:[N] int, gate-MLP weights
W1/b1/W2/b2:
    gate  = gelu(h @ W1 + b1) @ W2 + b2            (erf gelu)
    alpha = segment_softmax(gate, batch_vec)       (1024 segments)
    out   = segment_sum(alpha[:,None] * h)         -> [1024, 512]

Sharding: data-parallel over graphs. Core c owns graphs [128c, 128c+128)
and its contiguous node range (batch_vec sorted => segments never
straddle cores).

Per-core design (all engines overlapped, DMA+PE near roofline):
- Gate phase (A): z^T = W1^T @ h^T per 512-node supertile on the tensor
  engine; merged gelu over d-chunk pairs on ACT; the small gate matmuls
  (a1 @ W2) for supertile s are DEFERRED to supertile s+1 so the PE
  never stalls waiting on ACT.
- Host reorders nodes so that same-graph node PAIRS sit in two aligned
  SBUF tiles; DVE computes wh = e0*h0 + e1*h1 (idle engine), which
  HALVES the pooling matmul stream on the PE. Odd leftover nodes
  (<=1 per graph) go through the classic e-weighted one-hot path in
  LT tile(s). exp runs in chunks every ES supertiles so pooling is
  interleaved into the gate phase; softmax max-subtraction is skipped
  (gates are O(1), mathematically identical).
- pooled = msp^T @ wh accumulated in PSUM; denom = msp^T @ [e0 e1];
  out = pooled / denom.
"""

import os
from contextlib import ExitStack, nullcontext

import numpy as np

import concourse.bass as bass
import concourse.mybir as mybir
from concourse import bacc
import concourse.tile as tile
from concourse.bass_utils import run_bass_kernel_spmd

F32 = mybir.dt.float32
F16 = mybir.dt.float16

N_NODES = 100000
H = 512
NUM_GRAPHS = 1024
N_CORES = 8
G = NUM_GRAPHS // N_CORES   # graphs per core = 128
NP_DEFAULT = 12800          # gate-path padded nodes (25 supertiles of 512)
KC = H // 128               # contraction chunks = 4

ES = int(os.environ.get("AP_ES", "4"))          # exp chunk every ES supertiles
EXPMODE = os.environ.get("AP_EXPMODE", "table")  # "table" | "tanh"
HT_BUFS = int(os.environ.get("AP_HT_BUFS", "4"))
WH_BUFS = int(os.environ.get("AP_WH_BUFS", "8"))
A1_BUFS = int(os.environ.get("AP_A1_BUFS", "6"))


def _build(np_ht: int, PT: int, LT: int, reps: int = 1, ablate: str = "",
           merged: bool = True):
    """Build the per-core Bass program (SPMD: same program, per-core data)."""
    T = np_ht // 128
    S = np_ht // 512

    nc = bacc.Bacc("TRN2", target_bir_lowering=False, debug=False)

    ht_d = nc.dram_tensor("hT", [H, np_ht], F16, kind="ExternalInput")
    hpp_d = nc.dram_tensor("hpp", [PT * 256, H], F16, kind="ExternalInput")
    hps_d = nc.dram_tensor("hps", [LT * 128, H], F16, kind="ExternalInput")
    w1_d = nc.dram_tensor("W1", [H, H], F16, kind="ExternalInput")
    b1v_d = nc.dram_tensor("b1v", [128, KC], F32, kind="ExternalInput")
    w2_d = nc.dram_tensor("W2v", [128, 2 * KC], F16, kind="ExternalInput")
    b2t_d = nc.dram_tensor("b2t", [128, 2], F32, kind="ExternalInput")
    bv_d = nc.dram_tensor("bvrel", [128, T], F32, kind="ExternalInput")
    io_d = nc.dram_tensor("iota", [128, 128], F16, kind="ExternalInput")
    out_d = nc.dram_tensor("out", [G, H], F32, kind="ExternalOutput")

    gelu = mybir.ActivationFunctionType.Gelu
    expf = mybir.ActivationFunctionType.Exp
    eq = mybir.AluOpType.is_equal
    mul = mybir.AluOpType.mult
    addop = mybir.AluOpType.add

    with tile.TileContext(nc) as tc, ExitStack() as ctx:
        consts = ctx.enter_context(tc.tile_pool(name="consts", bufs=1))
        ht_pool = ctx.enter_context(tc.tile_pool(name="ht", bufs=HT_BUFS))
        a1_pool = ctx.enter_context(tc.tile_pool(name="a1", bufs=A1_BUFS))
        hpp_pool = ctx.enter_context(tc.tile_pool(name="hpp", bufs=PT))
        hps_pool = ctx.enter_context(tc.tile_pool(name="hps", bufs=LT))
        wh_pool = ctx.enter_context(tc.tile_pool(name="wh", bufs=WH_BUFS))
        msp_pool = ctx.enter_context(tc.tile_pool(name="msp", bufs=6))
        small = ctx.enter_context(tc.tile_pool(name="small", bufs=2))
        psz = ctx.enter_context(tc.tile_pool(name="psz", bufs=2, space="PSUM"))
        psg = ctx.enter_context(tc.tile_pool(name="psg", bufs=2, space="PSUM"))
        psp = ctx.enter_context(tc.tile_pool(name="psp", bufs=1, space="PSUM"))
        psd = ctx.enter_context(tc.tile_pool(name="psd", bufs=1, space="PSUM"))

        w1_sb = []
        for k in range(KC):
            t = consts.tile([128, H], F16, tag=f"w1_{k}")
            nc.sync.dma_start(out=t, in_=w1_d.ap()[k * 128:(k + 1) * 128, :])
            w1_sb.append(t)
        b1_sb = consts.tile([128, KC], F32, tag="b1")
        nc.sync.dma_start(out=b1_sb, in_=b1v_d.ap())
        w2_sb = consts.tile([128, 2 * KC], F16, tag="w2")
        nc.sync.dma_start(out=w2_sb, in_=w2_d.ap())
        b2_sb = consts.tile([128, 2], F32, tag="b2")
        nc.sync.dma_start(out=b2_sb, in_=b2t_d.ap())
        io_sb = consts.tile([128, 128], F16, tag="iota")
        nc.sync.dma_start(out=io_sb, in_=io_d.ap())
        bv_sb = consts.tile([128, T], F32, tag="bv")
        nc.sync.dma_start(out=bv_sb, in_=bv_d.ap())
        ones_sb = consts.tile([128, 2], F16, tag="ones")
        nc.vector.memset(ones_sb, 1.0)
        gate_sb = consts.tile([128, T], F32, tag="gate")
        e32 = consts.tile([128, T], F32, tag="e32")
        e16 = consts.tile([128, T], F16, tag="e16")

        ht4 = ht_d.ap().rearrange("(k p) (s n) -> s p k n", p=128, n=512)
        hpp4 = hpp_d.ap().rearrange("(pt i r) d -> pt r i d", i=2, r=128)
        hps3 = hps_d.ap().rearrange("(l r) d -> l r d", r=128)

        do_A = ablate not in ("noA", "dmaonly")
        do_C = ablate not in ("noC", "dmaonly")
        do_gate = ablate not in ("nogate", "noA", "noact", "dmaonly")
        do_act = ablate not in ("noact",)

        loop_cm = tc.For_i(0, reps, 1) if reps > 1 else nullcontext()
        with loop_cm:
            if do_C:
                psp_t = psp.tile([128, H], F32, tag="pp")
                psd_t = psd.tile([128, 2], F32, tag="pd")
            if not do_gate:
                nc.vector.memset(e32, 0.125)
                nc.vector.memset(e16, 0.125)

            state = {"pp_first": True, "pt_dma": 0, "wh_done": 0,
                     "mm_done": 0, "e_cols": 0}
            hpp_tiles = {}
            hps_tiles = {}
            a1_tiles = {}
            wh_tiles = {}
            ms_tiles = {}

            def emit_pair_dmas(n):
                for _ in range(n):
                    if state["pt_dma"] >= PT:
                        return
                    tl = hpp_pool.tile([128, 2, H], F16, tag="hpp")
                    nc.sync.dma_start(out=tl, in_=hpp4[state["pt_dma"]])
                    hpp_tiles[state["pt_dma"]] = tl
                    state["pt_dma"] += 1

            def emit_pg(s):
                pg = psg.tile([128, 2 * KC], F32, tag="pg")
                for nch in range(4):
                    for d in range(KC):
                        g2, jj = divmod(d, 2)
                        nc.tensor.matmul(
                            out=pg[:, 2 * nch:2 * nch + 2],
                            lhsT=a1_tiles[s][g2][:, jj, nch * 128:(nch + 1) * 128],
                            rhs=w2_sb[:, 2 * d:2 * d + 2],
                            start=(d == 0), stop=(d == KC - 1))
                nc.vector.tensor_copy(out=gate_sb[:, 4 * s:4 * s + 4],
                                      in_=pg[:, 0:2 * KC:2])
                del a1_tiles[s]

            def emit_exp(upto_col):
                if upto_col <= state["e_cols"]:
                    return
                a, b = state["e_cols"], upto_col
                if EXPMODE == "tanh":
                    # exp(g+b2) = (1+t)/(1-t), t = tanh((g+b2)/2); tanh is in
                    # the gelu act-table set => no ACT table reloads.
                    w = b - a
                    tp = small.tile([128, 4 * ES], F32, tag="tp")
                    nc.scalar.activation(
                        out=tp[:, 0:w], in_=gate_sb[:, a:b],
                        func=mybir.ActivationFunctionType.Tanh,
                        bias=b2_sb[:, 1:2], scale=0.5)
                    num = small.tile([128, 4 * ES], F32, tag="tnum")
                    nc.vector.tensor_scalar(
                        out=num[:, 0:w], in0=tp[:, 0:w], scalar1=1.0,
                        scalar2=None, op0=addop)
                    den = small.tile([128, 4 * ES], F32, tag="tden")
                    nc.vector.tensor_scalar(
                        out=den[:, 0:w], in0=tp[:, 0:w], scalar1=-1.0,
                        scalar2=1.0, op0=mul, op1=addop)
                    rec = small.tile([128, 4 * ES], F32, tag="trec")
                    nc.vector.reciprocal(out=rec[:, 0:w], in_=den[:, 0:w])
                    nc.vector.tensor_mul(out=e32[:, a:b], in0=num[:, 0:w],
                                         in1=rec[:, 0:w])
                else:
                    nc.scalar.activation(
                        out=e32[:, a:b], in_=gate_sb[:, a:b],
                        func=expf, bias=b2_sb[:, 0:1], scale=1.0)
                nc.vector.tensor_copy(out=e16[:, a:b], in_=e32[:, a:b])
                state["e_cols"] = upto_col

            def emit_C_wh(upto_pt):
                upto_pt = min(upto_pt, PT, state["e_cols"] // 2)
                for pt in range(state["wh_done"], upto_pt):
                    hb = hpp_tiles.pop(pt)
                    whE = wh_pool.tile([128, H], F16, tag="whE")
                    nc.vector.tensor_scalar(
                        out=whE, in0=hb[:, 0, :],
                        scalar1=e32[:, 2 * pt:2 * pt + 1], scalar2=None,
                        op0=mul)
                    whS = wh_pool.tile([128, H], F16, tag="whS")
                    nc.vector.scalar_tensor_tensor(
                        out=whS, in0=hb[:, 1, :],
                        scalar=e32[:, 2 * pt + 1:2 * pt + 2], in1=whE,
                        op0=mul, op1=addop)
                    mspt = msp_pool.tile([128, 128], F16, tag="msp")
                    nc.vector.tensor_scalar(
                        out=mspt, in0=io_sb,
                        scalar1=bv_sb[:, 2 * pt:2 * pt + 1], scalar2=None,
                        op0=eq)
                    wh_tiles[pt] = whS
                    ms_tiles[pt] = mspt
                state["wh_done"] = max(state["wh_done"], upto_pt)

            def emit_C_mm(upto_pt):
                upto_pt = min(upto_pt, state["wh_done"])
                for pt in range(state["mm_done"], upto_pt):
                    whS = wh_tiles.pop(pt)
                    mspt = ms_tiles.pop(pt)
                    first = state["pp_first"]
                    state["pp_first"] = False
                    nc.tensor.matmul(out=psp_t, lhsT=mspt, rhs=whS,
                                     start=first, stop=False)
                    nc.tensor.matmul(out=psd_t, lhsT=mspt,
                                     rhs=e16[:, 2 * pt:2 * pt + 2],
                                     start=first, stop=False)
                state["mm_done"] = upto_pt

            # ---------------- main supertile loop ----------------
            for s in range(S):
                htb = ht_pool.tile([128, KC, H], F16, tag="ht")
                nc.sync.dma_start(out=htb, in_=ht4[s])
                if s == 0:
                    for l in range(LT):
                        tl = hps_pool.tile([128, H], F16, tag="hps")
                        nc.sync.dma_start(out=tl, in_=hps3[l])
                        hps_tiles[l] = tl
                emit_pair_dmas(4 if s == 0 else 2)

                if do_A:
                    for g2 in range(2):
                        pz = psz.tile([128, 2, H], F32, tag="pz")
                        for jj in range(2):
                            d = 2 * g2 + jj
                            for k in range(KC):
                                nc.tensor.matmul(
                                    out=pz[:, jj, :],
                                    lhsT=w1_sb[k][:, d * 128:(d + 1) * 128],
                                    rhs=htb[:, k, :],
                                    start=(k == 0), stop=(k == KC - 1))
                        a1t = a1_pool.tile([128, 2, H], F16, tag="a1")
                        if not do_act:
                            pass
                        elif merged:
                            nc.scalar.activation(out=a1t, in_=pz, func=gelu,
                                                 bias=b1_sb[:, 0:1], scale=1.0)
                        else:
                            for jj in range(2):
                                d = 2 * g2 + jj
                                nc.scalar.activation(
                                    out=a1t[:, jj, :], in_=pz[:, jj, :],
                                    func=gelu, bias=b1_sb[:, d:d + 1],
                                    scale=1.0)
                        a1_tiles.setdefault(s, {})[g2] = a1t
                    if do_gate and s >= 1:
                        emit_pg(s - 1)
                if do_gate and s > 0 and s % ES == 0:
                    emit_exp(4 * s)
                if do_C:
                    emit_C_mm(state["wh_done"])
                    emit_C_wh(min(state["wh_done"] + 3, 2 * s))

            # ---------------- tail ----------------
            if do_A and do_gate:
                emit_pg(S - 1)
                emit_exp(T)
            if do_C:
                emit_pair_dmas(PT)
                emit_C_wh(PT)
                emit_C_mm(PT)
                for l in range(LT):
                    msl = msp_pool.tile([128, 128], F16, tag="msl")
                    col = 2 * PT + l
                    nc.vector.tensor_scalar(
                        out=msl, in0=io_sb, scalar1=bv_sb[:, col:col + 1],
                        scalar2=e32[:, col:col + 1], op0=eq, op1=mul)
                    last = (l == LT - 1)
                    nc.tensor.matmul(out=psp_t, lhsT=msl, rhs=hps_tiles[l],
                                     start=False, stop=last)
                    nc.tensor.matmul(out=psd_t[:, 0:1], lhsT=msl,
                                     rhs=ones_sb[:, 0:1], start=False,
                                     stop=last)
                pdsb = small.tile([128, 2], F32, tag="pdsb")
                nc.vector.tensor_copy(out=pdsb, in_=psd_t)
                dsum = small.tile([128, 1], F32, tag="dsum")
                nc.vector.tensor_add(out=dsum, in0=pdsb[:, 0:1],
                                     in1=pdsb[:, 1:2])
                dcl = small.tile([128, 1], F32, tag="dcl")
                nc.vector.tensor_scalar(out=dcl, in0=dsum, scalar1=1e-35,
                                        scalar2=None,
                                        op0=mybir.AluOpType.max)
                rec = small.tile([128, 1], F32, tag="rec")
                nc.vector.reciprocal(out=rec, in_=dcl)
                osb = small.tile([128, H], F32, tag="osb")
                nc.vector.tensor_scalar(out=osb, in0=psp_t,
                                        scalar1=rec[:, 0:1], scalar2=None,
                                        op0=mul)
            else:
                osb = small.tile([128, H], F32, tag="osb")
                nc.vector.memset(osb, 0.0)
            nc.sync.dma_start(out=out_d.ap(), in_=osb)

    nc.compile()
    return nc


def _plan(bv: np.ndarray) -> dict:
    """Node reordering plan: same-graph pairs + leftover singles, per core."""
    bv = np.asarray(bv).astype(np.int64)
    bounds = np.searchsorted(bv, np.arange(0, NUM_GRAPHS + 1, G))
    cores = []
    for c in range(N_CORES):
        n0, n1 = int(bounds[c]), int(bounds[c + 1])
        rel = bv[n0:n1] - c * G
        lens = np.bincount(rel, minlength=G)
        ends = np.cumsum(lens)
        starts = ends - lens
        p0_list, singles = [], []
        for g in range(G):
            ln = int(lens[g])
            s0 = int(starts[g])
            if ln >= 2:
                p0_list.append(s0 + 2 * np.arange(ln // 2))
            if ln % 2:
                singles.append(s0 + ln - 1)
        p0 = (np.concatenate(p0_list) if p0_list
              else np.empty(0, np.int64))
        sg = np.asarray(singles, np.int64)
        cores.append({"n0": n0, "n1": n1, "rel": rel, "p0": p0, "sg": sg})
    PT = max(1, -(-max(len(cc["p0"]) for cc in cores) // 128))
    LT = max(1, -(-max(len(cc["sg"]) for cc in cores) // 128))
    np_pool = PT * 256 + LT * 128
    np_ht = max(NP_DEFAULT, -(-np_pool // 512) * 512)
    return {"PT": PT, "LT": LT, "np_ht": np_ht, "cores": cores}


def _merged(b1: np.ndarray) -> bool:
    return bool(np.all(b1 == b1.reshape(-1)[0]))


def _prep_in_maps(h, bv, W1, b1, W2, b2, plan):
    """Shard + reorder + pad inputs per core; list of per-core input dicts."""
    PT, LT, np_ht = plan["PT"], plan["LT"], plan["np_ht"]
    T = np_ht // 128

    w1f = np.ascontiguousarray(W1.astype(np.float16))
    b1v = np.ascontiguousarray(
        b1.astype(np.float32).reshape(KC, 128).T)
    w2v = np.zeros((128, 2 * KC), np.float16)
    w2v[:, 0::2] = W2[:, 0].astype(np.float16).reshape(KC, 128).T
    b2s = np.float32(b2.reshape(-1)[0])
    b2t = np.stack([np.full(128, b2s, np.float32),
                    np.full(128, b2s * 0.5, np.float32)], axis=1)
    iota = np.ascontiguousarray(
        np.tile(np.arange(128, dtype=np.float16), (128, 1)))

    in_maps = []
    for cc in plan["cores"]:
        hc = h[cc["n0"]:cc["n1"]]
        rel = cc["rel"]
        p0, sg = cc["p0"], cc["sg"]
        m0 = np.full(PT * 128, -1, np.int64)
        m0[:len(p0)] = p0
        m1 = np.full(PT * 128, -1, np.int64)
        m1[:len(p0)] = p0 + 1
        pair_src = np.stack(
            [m0.reshape(PT, 128), m1.reshape(PT, 128)], axis=1).reshape(-1)
        sg_src = np.full(LT * 128, -1, np.int64)
        sg_src[:len(sg)] = sg
        src = np.concatenate([pair_src, sg_src])
        npool = len(src)

        hpool = np.zeros((np_ht, H), np.float32)
        valid = src >= 0
        hpool[:npool][valid] = hc[src[valid]]
        bvs = np.full(np_ht, -1.0, np.float32)
        bvs[:npool][valid] = rel[src[valid]].astype(np.float32)

        in_maps.append({
            "hT": np.ascontiguousarray(hpool.T.astype(np.float16)),
            "hpp": np.ascontiguousarray(hpool[:PT * 256].astype(np.float16)),
            "hps": np.ascontiguousarray(
                hpool[PT * 256:PT * 256 + LT * 128].astype(np.float16)),
            "W1": w1f,
            "b1v": b1v,
            "W2v": w2v,
            "b2t": b2t,
            "bvrel": np.ascontiguousarray(bvs.reshape(T, 128).T),
            "iota": iota,
        })
    return in_maps


_prog_cache: dict = {}


def _get_prog(np_ht, PT, LT, merged, reps=1, ablate=""):
    key = (np_ht, PT, LT, merged, reps, ablate, ES)
    if key not in _prog_cache:
        _prog_cache[key] = _build(np_ht, PT, LT, reps=reps, ablate=ablate,
                                  merged=merged)
    return _prog_cache[key]


def kernel(**inputs) -> np.ndarray:
    h = np.ascontiguousarray(np.asarray(inputs["h"], dtype=np.float32))
    bv = np.asarray(inputs["batch_vec"]).astype(np.int64)
    W1 = np.asarray(inputs["W1"], dtype=np.float32)
    b1 = np.asarray(inputs["b1"], dtype=np.float32)
    W2 = np.asarray(inputs["W2"], dtype=np.float32)
    b2 = np.asarray(inputs["b2"], dtype=np.float32)

    plan = _plan(bv)
    nc = _get_prog(plan["np_ht"], plan["PT"], plan["LT"], _merged(b1))
    in_maps = _prep_in_maps(h, bv, W1, b1, W2, b2, plan)
    trace = bool(int(os.environ.get("AP_TRACE", "0")))
    res = run_bass_kernel_spmd(nc, in_maps, list(range(N_CORES)), trace=trace)
    global last_results
    last_results = res
    out = np.concatenate([res.results[c]["out"] for c in range(N_CORES)],
                         axis=0).astype(np.float32)
    return out


last_results = None
